# revision 39
# baseline (speedup 1.0000x reference)
"""MoE head (router top-2 + per-expert GELU FFN) on 8 TRN2 NeuronCores.

Output DMAs issue on the ACT hwdge ring (nc.scalar.dma_start) so they
never queue behind the input-load stream on the SP ring — on one ring,
delayed y completions propagate WAR stalls (ybuf -> ps2) back to the PE
(~2.5 us per slot transition, found via TimelineSim gap analysis).
Token groups use the fewest splits possible (ceil(cap/512), ragged tail
>= 256): per-matmul fixed overhead is ~30-40 ns on HW, so slot B at
[512,492] (128 matmuls) beats a 128-aligned 3-way split (192 matmuls)
by ~2 us.

Strategy (sharding hint: expert parallel with top-k dispatch), default
mode "split" = expert parallel with pairwise DHID splitting:
  - Host computes the (tiny) router in float64: logits = x @ Wr.T,
    top-2 experts per token, softmax-over-top2 gate weights; tokens are
    gathered per expert on the host ("all-to-all dispatch").
  - Each core runs HALF the hidden dim (8 of 16 DHID d-blocks) for TWO
    experts: one of the 4 busiest (slot A, capA = their max count) and
    one of the 4 least busy (slot B, capB). Cores k and k+4 hold the
    two halves of the same expert pair; the host sums the two half-DHID
    partial y's. This cuts per-core moving columns from 128*maxcount to
    64*(capA+capB) (~4% for the reference routing) with the same matmul
    count and weight traffic — gelu is per-channel so the DHID split is
    exact.
  - Host applies the gate weight during its scatter-add of the two
    expert contributions per token (gate multiply is free there, so the
    device kernel has no DVE gating stage and no gate DMA).

  Device kernel (SPMD, one program, per-core data), all-bf16 I/O
  (host-cast; the 2e-2 rel-err budget dwarfs bf16's ~4e-3):
    mm1: h^T[dhid, tok] = W_in^T-chunks.T @ x^T-chunks  (PE, K=512,
         48 chains of 4 matmuls, moving dim = token groups of <=512)
    gelu: ACT engine, PSUM -> SBUF bf16
    mm2: y^T[dh, tok] = W_out-chunks.T @ h^T-chunks     (PE, K=2048,
         12 chains of 16, moving dim = token columns so the ragged
         last token-group costs proportionally, not a full subtile)
    ACT Copy drains PSUM->SBUF bf16 (measured lower PE contention than
    DVE: +29 vs +38 ns/matmul in stage mimics), DMA y^T out; host
    upcasts+transposes.

  Measured on HW (loop-slope, min-estimator): bf16 matmul sustains
  ~0.55 ns/col in-kernel (ACT/DVE PSUM-read contention + DVFS keep it
  above the 0.42 ns/col cost-model rate; TimelineSim shows 94% PE busy,
  so the schedule itself is clean). bf16 beats f32r (~1.4x) and fp8
  fails accuracy (~6% scale-rel err vs the 2e-2 gate). Total per-core
  DMA is 5.2 MB in + 1.1 MB out, fully hidden under compute when the
  input tiles are ping-pong double-buffered across iterations
  (persist bufs=2); per-kernel device time ~76 us vs the ~101 us f32r
  baseline. mm2-as-y^T (192 matmuls) measured faster than the
  token-subtile form (144 matmuls, more moving columns); multi-bank
  PSUM tiles (moving > 512) are rejected by the BIR verifier.
"""

import os

import numpy as np

P = 128
DH = 512
DHID = 2048
NE = 8
TOPK = 2
KC1 = DH // P  # k-chunks for mm1
KC2 = DHID // P  # k-chunks for mm2
DB2 = DH // P  # output d-blocks for mm2 (y^T partitions)
N_CORES = 8

_prog_cache: dict[tuple, object] = {}
LAST_EXEC_NS = None  # filled when MOE_TRACE=1
LAST_RESULTS = None
LAST_CAP = None
LAST_CAPS = None  # (capA, capB) in split mode


def _build_timing_program(loop_n: int):
    """Loop-timing program for test.py, matching the mode/caps of the last
    kernel() call (ping-pong, MOE_UNITS kernels per iteration)."""
    mode = _dtype_mode()
    if mode == "split":
        capA, capB = LAST_CAPS if LAST_CAPS else (1092, 1002)
        return _build_program_split(capA, capB, loop_n=loop_n, pingpong=True)
    return _build_program(
        LAST_CAP or 1092, mode, loop_n=loop_n, loop_scope="all", pingpong=True
    )


def _dtype_mode() -> str:
    return os.environ.get("MOE_DTYPE", "split")  # split | bf16h | f32r


def _groups_of(c: int) -> list[int]:
    """Token groups: each in [256, 512] so the matmul moving dim stays
    >= 256 (4x slower below). All groups except the last are multiples of
    128 so every group offset is 128-aligned; the last group absorbs any
    remainder."""
    if os.environ.get("MOE_G512", "0") == "1":
        gs = [512] * (c // 512)
        if c % 512:
            gs.append(c % 512)
        return gs
    if os.environ.get("MOE_GEVEN", "0") == "1" and c > 640:
        # near-even 3+ way split, multiples of 128 except the last
        n = -(-c // 512)
        base = (c // n) // 128 * 128
        gs = [base] * (n - 1)
        gs.append(c - base * (n - 1))
        return gs
    rem = c % 128
    if rem:
        last = 256 + rem  # in (256, 384)
        body = c - last
    else:
        last = None
        body = c
    gs = []
    left = body
    while left > 640:
        gs.append(512)
        left -= 512
    if left > 512:  # 640+128k leftovers: split into two >=256 chunks
        hi = (left // 256) * 128
        gs.extend([left - hi, hi])
    elif left:
        gs.append(left)
    if last is not None:
        gs.append(last)
    return gs


def _build_program(
    cap: int,
    mode: str,
    reps: int = 1,
    loop_n: int = 0,
    loop_scope: str = "all",
    pingpong: bool = False,
):
    """loop_n > 0 wraps the body in a runtime For_i loop (timing only).
    pingpong=True doubles the input tiles and puts TWO load+compute units
    in the loop body so iteration k+1's DMA overlaps iteration k's compute
    (steady-state pipelining; report slope/2 per kernel)."""
    import concourse.mybir as mybir
    import concourse.tile as tile
    from concourse import bacc

    f32 = mybir.dt.float32
    bf16 = mybir.dt.bfloat16
    mm_dt = bf16 if mode == "bf16h" else mybir.dt.float32r
    in_dt = mm_dt

    out_dt = bf16 if os.environ.get("MOE_YBF16", "1") == "1" and mode == "bf16h" else f32
    mm2_tok = os.environ.get("MOE_MM2", "yt") == "tok"
    nc = bacc.Bacc(None, target_bir_lowering=False, debug=False)
    xg = nc.declare_dram_parameter("xg", [DH, cap], in_dt, isOutput=False)
    w_in_t = nc.declare_dram_parameter("w_in_t", [DH, DHID], in_dt, isOutput=False)
    w_out = nc.declare_dram_parameter("w_out", [DHID, DH], in_dt, isOutput=False)
    if mm2_tok:
        # token-subtile mm2: y[tok, dh], 16-long chains, DH=512 moving
        yT = nc.declare_dram_parameter("y", [cap, DH], out_dt, isOutput=True)
    else:
        yT = nc.declare_dram_parameter("yT", [DH, cap], out_dt, isOutput=True)

    groups = _groups_of(cap)
    gelu = mybir.ActivationFunctionType.Gelu
    act_copy = mybir.ActivationFunctionType.Copy
    y_via_act = os.environ.get("MOE_YCOPY", "act") == "act"

    with tile.TileContext(nc) as tc:
        with (
            tc.tile_pool(name="persist", bufs=2 if pingpong else 1) as persist,
            tc.tile_pool(
                name="hbuf",
                bufs=int(os.environ.get("MOE_HBUF", "2" if cap <= 4608 else "1")),
            ) as hbuf,
            tc.tile_pool(name="ybuf", bufs=int(os.environ.get("MOE_YBUF", "4"))) as ybuf,
            tc.tile_pool(name="ps1", bufs=int(os.environ.get("MOE_PS1", "5")), space="PSUM") as ps1,
            tc.tile_pool(name="ps2", bufs=int(os.environ.get("MOE_PS2", "3")), space="PSUM") as ps2,
        ):
            state = {}
            g_offs = []
            o = 0
            for gsz in groups:
                g_offs.append((o, gsz))
                o += gsz

            def emit_loads():
                # First-use-ordered fine-grained input stream: the PE's
                # first work (mm1 group0, d0-3) needs only xg[:,g0] and
                # w_in quarter q0 -- land those first so the PE starts
                # a couple descriptors in instead of waiting on whole
                # chunks.
                xg_r = persist.tile([P, KC1, cap], in_dt, tag="xg_r")
                w_in_r = persist.tile([P, KC1, DHID], in_dt, tag="w_in_r")
                w_out_r = persist.tile([P, KC2, DH], in_dt, tag="w_out_r")
                Q = DHID // 4

                def dma_xg(kc, gi):
                    o, gsz = g_offs[gi]
                    nc.sync.dma_start(
                        out=xg_r[:, kc, o : o + gsz],
                        in_=xg[kc * P : (kc + 1) * P, o : o + gsz],
                    )

                def dma_win(kc, q):
                    nc.sync.dma_start(
                        out=w_in_r[:, kc, q * Q : (q + 1) * Q],
                        in_=w_in_t[kc * P : (kc + 1) * P, q * Q : (q + 1) * Q],
                    )

                for kc in range(KC1):
                    dma_xg(kc, 0)
                    dma_win(kc, 0)
                for kc in range(KC1):
                    dma_win(kc, 1)
                    dma_xg(kc, 1)
                for kc in range(KC1):
                    dma_win(kc, 2)
                for kc in range(KC1):
                    dma_win(kc, 3)
                for kc2 in range(KC2 // 2):
                    nc.sync.dma_start(
                        out=w_out_r[:, kc2 * 2 : (kc2 + 1) * 2, :],
                        in_=w_out.rearrange("(kc p) d -> p kc d", p=P)[
                            :, kc2 * 2 : (kc2 + 1) * 2, :
                        ],
                    )
                for gi in range(2, len(groups)):
                    for kc in range(KC1):
                        dma_xg(kc, gi)
                state.update(xg_r=xg_r, w_in_r=w_in_r, w_out_r=w_out_r)

            def emit_compute():
                # ---- mm1 -> gelu -> mm2 -> out, per token-group ----
                # Phase order is staggered (mm1 g0, mm1 g1, mm2 g0, mm1 g2,
                # mm2 g1, mm2 g2) so the first mm2 starts ~2 mm1-phases into
                # the kernel, giving the w_out DMA stream time to land
                # without stalling the PE. Needs 2 live h tiles (hbuf=2).
                xg_r, w_in_r = state["xg_r"], state["w_in_r"]
                w_out_r = state["w_out_r"]
                h_tiles = {}

                ilv = os.environ.get("MOE_ILV", "0") == "1"

                def mm1_phase(gi):
                    off, gsz = g_offs[gi]
                    h_r = hbuf.tile([P, KC2, gsz], mm_dt, tag="h_r")
                    h_tiles[gi] = h_r
                    if ilv:
                        # interleave pairs of accumulation chains so the PE
                        # always has an independent matmul to hide chain
                        # start/stop drains
                        for dp in range(KC2 // 2):
                            da, db = 2 * dp, 2 * dp + 1
                            pa = ps1.tile([P, gsz], f32, tag="p1")
                            pb = ps1.tile([P, gsz], f32, tag="p1")
                            for kc in range(KC1):
                                nc.tensor.matmul(
                                    pa,
                                    w_in_r[:, kc, da * P : (da + 1) * P],
                                    xg_r[:, kc, off : off + gsz],
                                    start=(kc == 0),
                                    stop=(kc == KC1 - 1),
                                )
                                nc.tensor.matmul(
                                    pb,
                                    w_in_r[:, kc, db * P : (db + 1) * P],
                                    xg_r[:, kc, off : off + gsz],
                                    start=(kc == 0),
                                    stop=(kc == KC1 - 1),
                                )
                            nc.scalar.activation(h_r[:, da, :], pa, gelu)
                            nc.scalar.activation(h_r[:, db, :], pb, gelu)
                        return
                    for d in range(KC2):
                        ps = ps1.tile([P, gsz], f32, tag="p1")
                        for kc in range(KC1):
                            nc.tensor.matmul(
                                ps,
                                w_in_r[:, kc, d * P : (d + 1) * P],
                                xg_r[:, kc, off : off + gsz],
                                start=(kc == 0),
                                stop=(kc == KC1 - 1),
                            )
                        nc.scalar.activation(h_r[:, d, :], ps, gelu)

                def mm2_chain(gi, db, h_r):
                    # one y^T d-block chain of gi (used by the mixed schedule)
                    off, gsz = g_offs[gi]
                    pt = ps2.tile([P, gsz], f32, tag="p2")
                    for d in range(KC2):
                        nc.tensor.matmul(
                            pt,
                            w_out_r[:, d, db * P : (db + 1) * P],
                            h_r[:, d, :],
                            start=(d == 0),
                            stop=(d == KC2 - 1),
                        )
                    y_sb = ybuf.tile([P, gsz], out_dt, tag="y_sb")
                    if os.environ.get("MOE_YCOPY", "act") == "split" and db % 2:
                        nc.vector.tensor_copy(y_sb, pt)
                    elif y_via_act:
                        nc.scalar.activation(y_sb, pt, act_copy)
                    else:
                        nc.vector.tensor_copy(y_sb, pt)
                    nc.sync.dma_start(
                        out=yT[db * P : (db + 1) * P, off : off + gsz], in_=y_sb
                    )

                def mm1_chain(gi, d, h_r):
                    off, gsz = g_offs[gi]
                    ps = ps1.tile([P, gsz], f32, tag="p1")
                    for kc in range(KC1):
                        nc.tensor.matmul(
                            ps,
                            w_in_r[:, kc, d * P : (d + 1) * P],
                            xg_r[:, kc, off : off + gsz],
                            start=(kc == 0),
                            stop=(kc == KC1 - 1),
                        )
                    nc.scalar.activation(h_r[:, d, :], ps, gelu)

                def mm2_phase(gi):
                    off, gsz = g_offs[gi]
                    h_r = h_tiles.pop(gi)
                    if mm2_tok:
                        for s in range(-(-gsz // P)):
                            m = min(P, gsz - s * P)
                            pt = ps2.tile([P, DH], f32, tag="p2")
                            for d in range(KC2):
                                nc.tensor.matmul(
                                    pt[:m, :],
                                    h_r[:, d, s * P : s * P + m],
                                    w_out_r[:, d, :],
                                    start=(d == 0),
                                    stop=(d == KC2 - 1),
                                )
                            tok0 = off + s * P
                            y_sb = ybuf.tile([P, DH], out_dt, tag="y_sb")
                            nc.vector.tensor_copy(y_sb[:m, :], pt[:m, :])
                            nc.sync.dma_start(
                                out=yT[tok0 : tok0 + m, :], in_=y_sb[:m, :]
                            )
                        return
                    for db in range(DB2):
                        pt = ps2.tile([P, gsz], f32, tag="p2")
                        for d in range(KC2):
                            nc.tensor.matmul(
                                pt,
                                w_out_r[:, d, db * P : (db + 1) * P],
                                h_r[:, d, :],
                                start=(d == 0),
                                stop=(d == KC2 - 1),
                            )
                        y_sb = ybuf.tile([P, gsz], out_dt, tag="y_sb")
                        if y_via_act:
                            nc.scalar.activation(y_sb, pt, act_copy)
                        else:
                            nc.vector.tensor_copy(y_sb, pt)
                        nc.sync.dma_start(
                            out=yT[db * P : (db + 1) * P, off : off + gsz], in_=y_sb
                        )

                n_g = len(groups)
                if os.environ.get("MOE_MIX", "0") == "1" and n_g >= 2:
                    # fine-grained software pipeline: interleave group gi's
                    # mm1 chains (16) with group gi-1's mm2 chains (4) at a
                    # 4:1 ratio, spreading ACT/DVE PSUM reads evenly instead
                    # of bunching per phase
                    mm1_phase(0)
                    for gi in range(1, n_g):
                        h_prev = h_tiles.pop(gi - 1)
                        off, gsz = g_offs[gi]
                        h_r = hbuf.tile([P, KC2, gsz], mm_dt, tag="h_r")
                        h_tiles[gi] = h_r
                        for d in range(KC2):
                            mm1_chain(gi, d, h_r)
                            if d % 4 == 3:
                                mm2_chain(gi - 1, d // 4, h_prev)
                    mm2_phase(n_g - 1)
                elif n_g == 1 or os.environ.get("MOE_STAGGER", "1") != "1":
                    for gi in range(n_g):
                        mm1_phase(gi)
                        mm2_phase(gi)
                else:
                    mm1_phase(0)
                    mm1_phase(1)
                    for gi in range(2, n_g):
                        mm2_phase(gi - 2)
                        mm1_phase(gi)
                    mm2_phase(n_g - 2)
                    mm2_phase(n_g - 1)

            units = int(os.environ.get("MOE_UNITS", "8")) if pingpong else 1
            if loop_n and loop_scope == "compute":
                emit_loads()
                with tc.For_i(0, loop_n, 1):
                    for _rep in range(reps):
                        emit_compute()
            elif loop_n and loop_scope == "loads":
                with tc.For_i(0, loop_n, 1):
                    for _rep in range(reps):
                        for _u in range(units):
                            emit_loads()
                emit_compute()
            elif loop_n:
                with tc.For_i(0, loop_n, 1):
                    for _rep in range(reps):
                        for _u in range(units):
                            emit_loads()
                            emit_compute()
            else:
                for _rep in range(reps):
                    emit_loads()
                    emit_compute()

    nc.compile()
    return nc


def _build_program_split(
    capA: int, capB: int, loop_n: int = 0, pingpong: bool = False, reps: int = 1
):
    """Pairwise DHID-split mode: each core runs HALF the hidden dim (8 of 16
    d-blocks) for one big expert (slot A, capA tokens) and one small expert
    (slot B, capB tokens). The host sums the two half-DHID partial y's per
    expert during its scatter-add. Per-core moving columns: 64*(capA+capB)
    vs 128*capA for expert-parallel — ~4% less for this distribution.
    PSUM/h/y tiles are allocated at fixed 512-col shapes and sliced, so the
    two slots' different group sizes don't multiply pool shapes."""
    import concourse.mybir as mybir
    import concourse.tile as tile
    from concourse import bacc

    f32 = mybir.dt.float32
    bf16 = mybir.dt.bfloat16
    KH = KC2 // 2  # d-blocks per half (8)

    nc = bacc.Bacc(None, target_bir_lowering=False, debug=False)
    prm = {}
    for s, cap in (("a", capA), ("b", capB)):
        prm[f"xg_{s}"] = nc.declare_dram_parameter(f"xg_{s}", [DH, cap], bf16, isOutput=False)
        prm[f"w_in_{s}"] = nc.declare_dram_parameter(f"w_in_{s}", [DH, KH * P], bf16, isOutput=False)
        prm[f"w_out_{s}"] = nc.declare_dram_parameter(f"w_out_{s}", [KH * P, DH], bf16, isOutput=False)
        prm[f"y_{s}"] = nc.declare_dram_parameter(f"y_{s}", [DH, cap], bf16, isOutput=True)

    gelu = mybir.ActivationFunctionType.Gelu
    act_copy = mybir.ActivationFunctionType.Copy
    caps = {"a": capA, "b": capB}

    def _min_groups(c):
        # fewest groups (fewest matmul instructions): n-1 full 512s + rest,
        # provided the ragged tail stays >= 256; else the 128-aligned split
        n = -(-c // 512)
        last = c - 512 * (n - 1)
        if n >= 2 and last >= 256:
            return [512] * (n - 1) + [last]
        return _groups_of(c)

    groups = {s: _min_groups(caps[s]) for s in ("a", "b")}
    g_offs = {}
    for s in ("a", "b"):
        o = 0
        g_offs[s] = []
        for gsz in groups[s]:
            g_offs[s].append((o, gsz))
            o += gsz

    with tile.TileContext(nc) as tc:
        with (
            tc.tile_pool(name="persist", bufs=2 if pingpong else 1) as persist,
            tc.tile_pool(name="hbuf", bufs=2) as hbuf,
            tc.tile_pool(name="ybuf", bufs=int(os.environ.get("MOE_YBUF2", "4"))) as ybuf,
            tc.tile_pool(name="ps1", bufs=int(os.environ.get("MOE_PS1S", "5")), space="PSUM") as ps1,
            tc.tile_pool(name="ps2", bufs=int(os.environ.get("MOE_PS2S", "3")), space="PSUM") as ps2,
        ):
            state = {}

            def emit_loads():
                Q = KH * P // 2
                for s in ("a", "b"):
                    cap = caps[s]
                    xg_r = persist.tile([P, KC1, cap], bf16, tag=f"xg_{s}")
                    w_in_r = persist.tile([P, KC1, KH * P], bf16, tag=f"wi_{s}")
                    w_out_r = persist.tile([P, KH, DH], bf16, tag=f"wo_{s}")
                    state.update({f"xg_{s}": xg_r, f"wi_{s}": w_in_r, f"wo_{s}": w_out_r})
                for s in ("a", "b"):
                    xg_r, w_in_r, w_out_r = state[f"xg_{s}"], state[f"wi_{s}"], state[f"wo_{s}"]
                    xg, w_in_t, w_out = prm[f"xg_{s}"], prm[f"w_in_{s}"], prm[f"w_out_{s}"]
                    for gi, (o, gsz) in enumerate(g_offs[s][:2]):
                        for kc in range(KC1):
                            nc.sync.dma_start(
                                out=xg_r[:, kc, o : o + gsz],
                                in_=xg[kc * P : (kc + 1) * P, o : o + gsz],
                            )
                            if gi == 0:
                                nc.sync.dma_start(
                                    out=w_in_r[:, kc, gi * Q : (gi + 1) * Q],
                                    in_=w_in_t[kc * P : (kc + 1) * P, gi * Q : (gi + 1) * Q],
                                )
                    for kc in range(KC1):
                        nc.sync.dma_start(
                            out=w_in_r[:, kc, Q : 2 * Q],
                            in_=w_in_t[kc * P : (kc + 1) * P, Q : 2 * Q],
                        )
                    for kc2 in range(KH // 2):
                        nc.sync.dma_start(
                            out=w_out_r[:, kc2 * 2 : (kc2 + 1) * 2, :],
                            in_=w_out.rearrange("(kc p) d -> p kc d", p=P)[
                                :, kc2 * 2 : (kc2 + 1) * 2, :
                            ],
                        )
                    for o, gsz in g_offs[s][2:]:
                        for kc in range(KC1):
                            nc.sync.dma_start(
                                out=xg_r[:, kc, o : o + gsz],
                                in_=xg[kc * P : (kc + 1) * P, o : o + gsz],
                            )

            def emit_compute():
                for s in ("a", "b"):
                    xg_r, w_in_r, w_out_r = state[f"xg_{s}"], state[f"wi_{s}"], state[f"wo_{s}"]
                    yT = prm[f"y_{s}"]
                    h_tiles = {}

                    def mm1_phase(gi):
                        off, gsz = g_offs[s][gi]
                        h_r = hbuf.tile([P, KH, 512], bf16, tag="h_r")
                        h_tiles[gi] = h_r
                        for d in range(KH):
                            ps = ps1.tile([P, 512], f32, tag="p1")
                            for kc in range(KC1):
                                nc.tensor.matmul(
                                    ps[:, :gsz],
                                    w_in_r[:, kc, d * P : (d + 1) * P],
                                    xg_r[:, kc, off : off + gsz],
                                    start=(kc == 0),
                                    stop=(kc == KC1 - 1),
                                )
                            nc.scalar.activation(h_r[:, d, :gsz], ps[:, :gsz], gelu)

                    def mm2_phase(gi):
                        off, gsz = g_offs[s][gi]
                        h_r = h_tiles.pop(gi)
                        for db in range(DB2):
                            pt = ps2.tile([P, 512], f32, tag="p2")
                            for d in range(KH):
                                nc.tensor.matmul(
                                    pt[:, :gsz],
                                    w_out_r[:, d, db * P : (db + 1) * P],
                                    h_r[:, d, :gsz],
                                    start=(d == 0),
                                    stop=(d == KH - 1),
                                )
                            y_sb = ybuf.tile([P, 512], bf16, tag="y_sb")
                            # y-drain on DVE: ACT must stay clear for the next
                            # slot's gelus — queueing y copies on ACT stalls
                            # the PE ~2.6us at each slot transition (sim)
                            yc = os.environ.get("MOE_YCOPY", "dve")
                            if yc == "act":
                                nc.scalar.activation(y_sb[:, :gsz], pt[:, :gsz], act_copy)
                            elif yc == "pool":
                                # Pool engine is otherwise idle in this kernel
                                nc.gpsimd.tensor_copy(y_sb[:, :gsz], pt[:, :gsz])
                            else:
                                nc.vector.tensor_copy(y_sb[:, :gsz], pt[:, :gsz])
                            # y-out on the ACT hwdge ring: keeps output DMAs
                            # out of the input-load ring (SP), whose multi-MB
                            # backlog otherwise delays y completion -> ybuf/
                            # ps2 WAR stalls the PE mid-mm2
                            dma_eng = (
                                nc.scalar
                                if os.environ.get("MOE_YQ", "act") == "act"
                                else nc.sync
                            )
                            dma_eng.dma_start(
                                out=yT[db * P : (db + 1) * P, off : off + gsz],
                                in_=y_sb[:, :gsz],
                            )

                    n_g = len(groups[s])
                    mm1_phase(0)
                    mm1_phase(1)
                    for gi in range(2, n_g):
                        mm2_phase(gi - 2)
                        mm1_phase(gi)
                    mm2_phase(n_g - 2)
                    mm2_phase(n_g - 1)

            units = int(os.environ.get("MOE_UNITS", "8")) if pingpong else 1
            if loop_n:
                with tc.For_i(0, loop_n, 1):
                    for _u in range(units):
                        emit_loads()
                        emit_compute()
            else:
                for _r in range(reps):
                    emit_loads()
                    emit_compute()

    nc.compile()
    return nc


def _get_program(cap: int, mode: str):
    key = (cap, mode)
    if key not in _prog_cache:
        _prog_cache[key] = _build_program(cap, mode)
    return _prog_cache[key]


def kernel(x, Wr, W_in, W_out):
    global LAST_EXEC_NS, LAST_RESULTS, LAST_CAP
    import ml_dtypes
    from concourse.bass_utils import run_bass_kernel_spmd

    bf16 = np.dtype(ml_dtypes.bfloat16)
    x = np.ascontiguousarray(np.asarray(x), dtype=np.float32)
    Wr = np.asarray(Wr, dtype=np.float32)
    W_in = np.asarray(W_in, dtype=np.float32)
    W_out = np.asarray(W_out, dtype=np.float32)
    T = x.shape[0]

    # ---- host router (fp64: strictly more accurate than the fp32 ref) ----
    logits = x.astype(np.float64) @ Wr.astype(np.float64).T  # (T, NE)
    part = np.argpartition(-logits, TOPK - 1, axis=1)[:, :TOPK]
    vals = np.take_along_axis(logits, part, axis=1)
    order = np.argsort(-vals, axis=1, kind="stable")
    idx = np.take_along_axis(part, order, axis=1)  # (T, 2) desc
    ar = np.arange(T)
    v1 = logits[ar, idx[:, 0]]
    v2 = logits[ar, idx[:, 1]]
    e2 = np.exp(v2 - v1)
    w1 = (1.0 / (1.0 + e2)).astype(np.float32)
    w2 = (e2 / (1.0 + e2)).astype(np.float32)

    tok_lists, gate_lists = [], []
    for e in range(NE):
        s1 = np.nonzero(idx[:, 0] == e)[0]
        s2 = np.nonzero(idx[:, 1] == e)[0]
        tok_lists.append(np.concatenate([s1, s2]))
        gate_lists.append(np.concatenate([w1[s1], w2[s2]]))
    max_count = max(len(t) for t in tok_lists)

    mode = _dtype_mode()
    if mode == "split":
        return _kernel_split(x, W_in, W_out, tok_lists, gate_lists)

    cap_env = os.environ.get("MOE_CAP")
    cap = int(cap_env) if cap_env else -(-max_count // 4) * 4
    if max_count > cap:
        cap = -(-max_count // 4) * 4
    cap = max(cap, 384)
    LAST_CAP = cap
    nc = _get_program(cap, mode)

    in_np_dt = bf16 if mode == "bf16h" else np.float32
    xc = x.astype(in_np_dt) if mode == "bf16h" else x
    in_maps = []
    for e in range(NE):
        toks = tok_lists[e]
        xg = np.zeros((DH, cap), in_np_dt)
        xg[:, : len(toks)] = xc[toks].T
        in_maps.append(
            {
                "xg": xg,
                "w_in_t": np.ascontiguousarray(W_in[e].T.astype(in_np_dt)),
                "w_out": np.ascontiguousarray(W_out[e].astype(in_np_dt)),
            }
        )

    trace = os.environ.get("MOE_TRACE", "0") == "1"
    res = run_bass_kernel_spmd(
        nc,
        in_maps,
        list(range(N_CORES)),
        trace=trace,
        trace_cores=list(range(N_CORES)) if trace else None,
    )
    LAST_EXEC_NS = res.exec_time_ns
    LAST_RESULTS = res

    out = np.zeros((T, DH), np.float32)
    for e in range(NE):
        toks = tok_lists[e]
        if len(toks):
            r = res.results[e]
            if "y" in r:
                ye = r["y"][: len(toks), :].astype(np.float32)
            else:
                ye = r["yT"][:, : len(toks)].T.astype(np.float32)
            out[toks] += gate_lists[e][:, None] * ye
    return out


def _kernel_split(x, W_in, W_out, tok_lists, gate_lists):
    """Pairwise DHID-split dispatch: core k (k=0..3 half0, k+4 half1) runs
    d-blocks of one big expert (slot A) and one small expert (slot B); the
    host sums the two half-DHID partial y's per expert into the scatter-add."""
    global LAST_EXEC_NS, LAST_RESULTS, LAST_CAP, LAST_CAPS
    import ml_dtypes
    from concourse.bass_utils import run_bass_kernel_spmd

    bf16 = np.dtype(ml_dtypes.bfloat16)
    T = x.shape[0]
    counts = np.array([len(t) for t in tok_lists])
    order = np.argsort(-counts, kind="stable")
    bigs, smalls = order[:4], order[4:]
    capA = max(-(-int(counts[bigs[0]]) // 4) * 4, 384)
    capB = max(-(-int(counts[smalls[0]]) // 4) * 4, 384)
    LAST_CAP = capA
    LAST_CAPS = (capA, capB)

    key = ("split", capA, capB)
    if key not in _prog_cache:
        _prog_cache[key] = _build_program_split(capA, capB)
    nc = _prog_cache[key]

    xc = x.astype(bf16)
    HALF = DHID // 2

    def slot_inputs(e, half, cap):
        toks = tok_lists[e]
        xg = np.zeros((DH, cap), bf16)
        xg[:, : len(toks)] = xc[toks].T
        rows = slice(half * HALF, (half + 1) * HALF)
        return (
            xg,
            np.ascontiguousarray(W_in[e][rows, :].T.astype(bf16)),
            np.ascontiguousarray(W_out[e][rows, :].astype(bf16)),
        )

    in_maps = []
    for k in range(N_CORES):
        half, pi = divmod(k, 4)[0], k % 4
        eA, eB = int(bigs[pi]), int(smalls[pi])
        xga, wia, woa = slot_inputs(eA, half, capA)
        xgb, wib, wob = slot_inputs(eB, half, capB)
        in_maps.append(
            {
                "xg_a": xga, "w_in_a": wia, "w_out_a": woa,
                "xg_b": xgb, "w_in_b": wib, "w_out_b": wob,
            }
        )

    trace = os.environ.get("MOE_TRACE", "0") == "1"
    res = run_bass_kernel_spmd(
        nc,
        in_maps,
        list(range(N_CORES)),
        trace=trace,
        trace_cores=list(range(N_CORES)) if trace else None,
    )
    LAST_EXEC_NS = res.exec_time_ns
    LAST_RESULTS = res

    out = np.zeros((T, DH), np.float32)
    for pi in range(4):
        for slot, e in (("y_a", int(bigs[pi])), ("y_b", int(smalls[pi]))):
            toks = tok_lists[e]
            if not len(toks):
                continue
            yT = res.results[pi][slot].astype(np.float32) + res.results[pi + 4][
                slot
            ].astype(np.float32)
            out[toks] += gate_lists[e][:, None] * yT[:, : len(toks)].T
    return out
